# revision 31
# baseline (speedup 1.0000x reference)
"""GPT-J attention (B=2, S=2048, D=4096, 16 heads x 256, partial RoPE 64) on 8 trn2 cores.

Sharding: DP x TP = 2 batches x 4 head-groups. Core c owns batch c//4 and
heads [4*(c%4), 4*(c%4)+4) (Wq/Wk/Wv column slices of 1024, Wo row slice).
Each core computes its 4 heads' attention for its batch, then the partial
out-projection [2048, 4096]; a 4-core ReduceScatter (per column chunk,
overlapped with the out-projection) sums partials and leaves each core with
a [512, col-chunk] shard; host reassembles.

Per-core kernel (all bf16 matmuls, fp32 PSUM accumulation):
  - phase A (x2 head-groups hg of 512 cols): QKV projection with d streamed
    in 4 groups of 1024; partials accumulated into SBUF bf16 (DVE).
    hsT/weights are host-pre-tiled so every DMA is 128 contiguous
    per-partition descriptors.
  - RoPE via pair-swap PE matmul + DVE muls on the first 64 rows of each
    head (cos/sin gathered per position on host).
  - attention (per head, 512-query macro tiles): scores computed
    TRANSPOSED (k on partitions, q on free) so exp'd probs PT feed PV
    directly with no PE transpose / PSUM->SBUF copy. Rowsums via a
    ones-vector matmul into psum row 0, reciprocal on DVE, broadcast to
    128 partitions on Pool (gpsimd), applied at ATN evacuation (DVE).
  - phase C: out-proj accumulates all 8 ATN chunks in one PSUM chain;
    evacuation alternates DVE/ACT to bf16; ReduceScatter issued per
    1024-column chunk so the collective overlaps the remaining compute.
"""

import os
import sys

import numpy as np

sys.path.insert(0, "/opt/trn_rl_repo")

# ---------------------------------------------------------------- constants
B = 2
S = 2048
D = 4096
NH = 16
HD = 256
ROT = 64
MAX_POS = 2048
N_CORES = 8
GRP = 4                      # replica-group size (TP groups per batch)
HPC = NH // GRP              # heads per core = 4
HDL = HPC * HD               # local head width = 1024
NHG = 2                      # head-groups per core
HGW = HDL // NHG             # head-group width = 512
NHC = HGW // 128             # chunks per head-group = 4
NG = 4                       # d-groups
DG = D // NG                 # 1024
GDC = DG // 128              # 8
SC = 512                     # s-chunk
NSC = S // SC                # 4
NOC = D // SC                # out-proj column strips = 8
NCH = 4                      # collective chunks (pairs of strips)
CW = D // NCH                # 1024 cols per chunk
SHARD = S // GRP             # 512 rows per core shard
NEG = -1.0e30


def _cfg_full():
    return dict(B=B, S=S, D=D, HPC=HPC, HD=HD, ROT=ROT)


# ---------------------------------------------------------------- bass build

def build_nc(use_collective=True, n_cores=N_CORES, mm_dtype="bfloat16",
             debug_taps=False):
    import concourse.tile as tile
    from concourse import bacc, mybir

    fp32 = mybir.dt.float32
    mdt = getattr(mybir.dt, mm_dtype)

    nc = bacc.Bacc(num_devices=n_cores)

    # per-core inputs (host-pre-tiled for contiguous per-partition DMA)
    hsT_e = nc.declare_dram_parameter("hsT", [NG, 2, 128, GDC, S // 2], mdt,
                                      isOutput=False)
    wq_e = nc.declare_dram_parameter("wq", [NHG, NG, 128, GDC, HGW], mdt,
                                     isOutput=False)
    wk_e = nc.declare_dram_parameter("wk", [NHG, NG, 128, GDC, HGW], mdt,
                                     isOutput=False)
    wv_e = nc.declare_dram_parameter("wv", [NHG, NG, 128, GDC, HGW], mdt,
                                     isOutput=False)
    wo_e = nc.declare_dram_parameter("wo", [NOC, 128, HDL // 128, SC], mdt,
                                     isOutput=False)
    cos_e = nc.declare_dram_parameter("cosb", [ROT, S], mdt, isOutput=False)
    sin_e = nc.declare_dram_parameter("sinb", [ROT, S], mdt, isOutput=False)
    mskT_e = nc.declare_dram_parameter("maskT", [128, 4, SC], fp32,
                                       isOutput=False)
    psw_e = nc.declare_dram_parameter("pswap", [128, ROT], mdt, isOutput=False)

    # collective row-chunks: issued after the sg-block that completes them.
    # decreasing sizes near the end so the CC pipeline drains with compute.
    RS_CHUNKS = [(0, 512), (512, 896), (896, 1280), (1280, 1664),
                 (1664, 1920), (1920, 2048)]
    if use_collective:
        y_e = nc.declare_dram_parameter("y", [S // GRP, D], mdt,
                                        isOutput=True)
        y_part = nc.dram_tensor("y_part", [S, D], mdt)
        rs_out = nc.dram_tensor("rs_out", [S // GRP, D], mdt)
        cc_warm_in = nc.dram_tensor("cc_warm_in", [GRP * 128], mdt)
        cc_warm_out = nc.dram_tensor("cc_warm_out", [128], mdt)
        rgroups = [[0, 1, 2, 3], [4, 5, 6, 7]]
    else:
        y_e = nc.declare_dram_parameter("y", [S, D], mdt, isOutput=True)

    if debug_taps:
        dbg = {}
        for nm, shp in [("dbg_qt", [NHG, NHC, 128, S]),
                        ("dbg_kt", [NHG, NHC, 128, S]),
                        ("dbg_v", [NHG, S, HGW]),
                        ("dbg_atn", [NHG * NHC, 128, S])]:
            dbg[nm] = nc.declare_dram_parameter(nm, shp, mdt, isOutput=True)

    def mm(ps, lhsT, rhs, start, stop):
        nc.tensor.matmul(ps, lhsT, rhs, start=start, stop=stop)

    with tile.TileContext(nc) as tc:
        with tc.tile_pool(name="const", bufs=1) as constp:
            mskT = constp.tile([128, 4, SC], fp32)
            nc.sync.dma_start(mskT[:], mskT_e[:])
            pswap = constp.tile([128, ROT], mdt)
            nc.sync.dma_start(pswap[:], psw_e[:])
            onesq = constp.tile([128, 128], mdt)
            nc.vector.memset(onesq[:], 1.0)
            cosb = constp.tile([ROT, S], mdt)
            sinb = constp.tile([ROT, S], mdt)
            nc.sync.dma_start(cosb[:], cos_e[:])
            nc.sync.dma_start(sinb[:], sin_e[:])
            if use_collective:
                # tiny warmup collective: absorbs mesh-algo init cost while
                # phase A runs (input zero-filled first — garbage bits can
                # be NaN and upset the CC cores)
                warmz = constp.tile([128, GRP], mdt)
                nc.vector.memset(warmz[:], 0.0)
                nc.sync.dma_start(
                    cc_warm_in[:].rearrange("(p g) -> p g", p=128), warmz[:])
                nc.gpsimd.collective_compute(
                    "ReduceScatter", mybir.AluOpType.add,
                    replica_groups=rgroups,
                    ins=[cc_warm_in[:]], outs=[cc_warm_out[:]])

            with (
                tc.tile_pool(name="qkv", bufs=1) as qkvp,
                tc.tile_pool(name="atn", bufs=1) as atnp,
            ):
                ATN = [atnp.tile([128, S], mdt, tag=f"ATN{c}", name=f"ATN{c}")
                       for c in range(NHG * NHC)]
                for hg in range(NHG):
                    QT = [qkvp.tile([128, S], mdt, tag=f"QT{c}", name=f"QT{c}")
                          for c in range(NHC)]
                    KT = [qkvp.tile([128, S], mdt, tag=f"KT{c}", name=f"KT{c}")
                          for c in range(NHC)]
                    V = [qkvp.tile([128, HGW], mdt, tag=f"V{ss}", name=f"V{ss}")
                         for ss in range(S // 128)]

                    # ---------------- phase A: QKV projection ----------------
                    with (
                        tc.tile_pool(name="wts", bufs=1) as wp,
                        tc.tile_pool(name="hst", bufs=2) as hp,
                        tc.tile_pool(name="pjps", bufs=1, space="PSUM") as pjps,
                    ):
                        for g in range(NG):
                            wq = wp.tile([128, GDC, HGW], mdt, tag="wq")
                            wk = wp.tile([128, GDC, HGW], mdt, tag="wk")
                            wv = wp.tile([128, GDC, HGW], mdt, tag="wv")
                            nc.sync.dma_start(wq[:], wq_e[hg, g])
                            nc.sync.dma_start(wk[:], wk_e[hg, g])
                            nc.sync.dma_start(wv[:], wv_e[hg, g])
                            for sh in range(2):
                                hst = hp.tile([128, GDC, S // 2], mdt, tag="hst")
                                nc.sync.dma_start(hst[:], hsT_e[g, sh])
                                for sc2 in range(2):
                                    sc = sh * 2 + sc2
                                    ssl = slice(sc * SC, (sc + 1) * SC)
                                    hsl = slice(sc2 * SC, (sc2 + 1) * SC)
                                    psq = [pjps.tile([128, SC], fp32,
                                                     tag=f"psq{h}", name=f"psq{h}")
                                           for h in range(NHC)]
                                    psk = [pjps.tile([128, SC], fp32,
                                                     tag=f"psk{h}", name=f"psk{h}")
                                           for h in range(NHC)]
                                    for dc in range(GDC):
                                        for h in range(NHC):
                                            osl = slice(h * 128, (h + 1) * 128)
                                            mm(psq[h][:], wq[:, dc, osl],
                                               hst[:, dc, hsl],
                                               start=(dc == 0), stop=(dc == GDC - 1))
                                            mm(psk[h][:], wk[:, dc, osl],
                                               hst[:, dc, hsl],
                                               start=(dc == 0), stop=(dc == GDC - 1))
                                    for h in range(NHC):
                                        if g == 0:
                                            nc.vector.tensor_copy(QT[h][:, ssl], psq[h][:])
                                            nc.vector.tensor_copy(KT[h][:, ssl], psk[h][:])
                                        else:
                                            nc.vector.tensor_add(
                                                QT[h][:, ssl], QT[h][:, ssl], psq[h][:])
                                            nc.vector.tensor_add(
                                                KT[h][:, ssl], KT[h][:, ssl], psk[h][:])
                                    psv = [pjps.tile([128, HGW], fp32,
                                                     tag=f"psq{ss}", name=f"psv{ss}")
                                           for ss in range(SC // 128)]
                                    for dc in range(GDC):
                                        for ss in range(SC // 128):
                                            ssub = slice(sc2 * SC + ss * 128,
                                                         sc2 * SC + (ss + 1) * 128)
                                            mm(psv[ss][:], hst[:, dc, ssub],
                                               wv[:, dc, :],
                                               start=(dc == 0), stop=(dc == GDC - 1))
                                    for ss in range(SC // 128):
                                        vi = sc * (SC // 128) + ss
                                        if g == 0:
                                            nc.scalar.copy(V[vi][:], psv[ss][:])
                                        else:
                                            nc.vector.tensor_add(
                                                V[vi][:], V[vi][:], psv[ss][:])

                    # ---------------- RoPE on QT/KT rot rows ----------------
                    with (
                        tc.tile_pool(name="rope", bufs=4) as ropep,
                        tc.tile_pool(name="rops", bufs=2, space="PSUM") as ropsp,
                    ):
                        for t in (QT, KT):
                            for hch in range(0, NHC, HD // 128):
                                for sc in range(NSC):
                                    ssl = slice(sc * SC, (sc + 1) * SC)
                                    sw = ropsp.tile([ROT, SC], fp32, tag="sw")
                                    mm(sw[:], pswap[:, :], t[hch][:, ssl],
                                       start=True, stop=True)
                                    t1 = ropep.tile([ROT, SC], mdt, tag="t1")
                                    t2 = ropep.tile([ROT, SC], mdt, tag="t2")
                                    nc.vector.tensor_tensor(
                                        t1[:], sw[:], sinb[:, ssl],
                                        op=mybir.AluOpType.mult)
                                    nc.vector.tensor_tensor(
                                        t2[:], t[hch][0:ROT, ssl], cosb[:, ssl],
                                        op=mybir.AluOpType.mult)
                                    nc.vector.tensor_add(t[hch][0:ROT, ssl],
                                                         t1[:], t2[:])

                    # ---------------- phase B: attention (transposed) --------
                    def attn_qm(pools, hg, qm):
                        ptp, rcpp, scps, atps, rsps = pools
                        qsl = slice(qm * SC, (qm + 1) * SC)
                        nkc = (qm + 1) * (SC // 128)
                        for h in range(HPC // NHG):
                            c0 = h * (HD // 128)
                            atn_ps = [atps.tile([128, SC], fp32,
                                                tag=f"atn{hh}", name=f"atn{hh}")
                                      for hh in range(HD // 128)]
                            rs_ps = rsps.tile([128, SC], fp32, tag="rs")
                            for kc in range(nkc):
                                kcl = slice(kc * 128, (kc + 1) * 128)
                                sT = scps.tile([128, SC], fp32, tag="sT")
                                mm(sT[:], KT[c0][:, kcl], QT[c0][:, qsl],
                                   start=True, stop=False)
                                mm(sT[:], KT[c0 + 1][:, kcl], QT[c0 + 1][:, qsl],
                                   start=False, stop=True)
                                if kc >= nkc - 4:
                                    nc.vector.tensor_add(
                                        sT[:], sT[:], mskT[:, kc - (nkc - 4), :])
                                PT = ptp.tile([128, SC], mdt, tag="PT")
                                nc.scalar.activation(
                                    PT[:], sT[:],
                                    mybir.ActivationFunctionType.Exp,
                                    bias=0.0, scale=1.0 / 16.0)
                                for hh in range(HD // 128):
                                    vsl = slice(h * HD + hh * 128,
                                                h * HD + (hh + 1) * 128)
                                    mm(atn_ps[hh][:], V[kc][:, vsl], PT[:],
                                       start=(kc == 0), stop=(kc == nkc - 1))
                                mm(rs_ps[:], onesq[:, :], PT[:],
                                   start=(kc == 0), stop=(kc == nkc - 1))
                            rcp = rcpp.tile([128, SC], fp32, tag="rcp")
                            # rowsums are in [1, ~2e7]: safe for the approx
                            nc.vector.reciprocal_approx_fast(rcp[:], rs_ps[:])
                            for hh in range(HD // 128):
                                nc.vector.tensor_tensor(
                                    ATN[hg * NHC + c0 + hh][:, qsl],
                                    atn_ps[hh][:], rcp[:],
                                    op=mybir.AluOpType.mult)

                    NAC = NHG * NHC      # 8 ATN chunks
                    if hg == 0:
                        with (
                            tc.tile_pool(name="ptsb", bufs=3) as ptp,
                            tc.tile_pool(name="rcp", bufs=2) as rcpp,
                            tc.tile_pool(name="scps", bufs=2, space="PSUM") as scps,
                            tc.tile_pool(name="atps", bufs=1, space="PSUM") as atps,
                            tc.tile_pool(name="rsps", bufs=1, space="PSUM") as rsps,
                        ):
                            for qm in range(NSC):
                                attn_qm((ptp, rcpp, scps, atps, rsps), 0, qm)
                    else:
                        # fused: attention(hg1, qm) -> out-proj rows 4qm..4qm+3
                        # -> ReduceScatter chunk qm (overlaps next qm compute)
                        with (
                            tc.tile_pool(name="ptsb", bufs=3) as ptp,
                            tc.tile_pool(name="rcp", bufs=2) as rcpp,
                            tc.tile_pool(name="wo", bufs=1) as wop,
                            tc.tile_pool(name="ysb", bufs=3) as ysbp,
                            tc.tile_pool(name="scps", bufs=2, space="PSUM") as scps,
                            tc.tile_pool(name="atps", bufs=1, space="PSUM") as atps,
                            tc.tile_pool(name="rsps", bufs=1, space="PSUM") as rsps,
                            tc.tile_pool(name="yps", bufs=3, space="PSUM") as ypsp,
                        ):
                            woc = [wop.tile([128, NAC, SC], mdt, tag=f"woc{oc}",
                                            name=f"woc{oc}") for oc in range(NOC)]
                            for oc in range(NOC):
                                nc.sync.dma_start(woc[oc][:], wo_e[oc])
                            dst = y_part if use_collective else y_e
                            for qm in range(NSC):
                                attn_qm((ptp, rcpp, scps, atps, rsps), 1, qm)
                                for sg in range(qm * 4, qm * 4 + 4):
                                    sgl = slice(sg * 128, (sg + 1) * 128)
                                    ysb = ysbp.tile([128, D], mdt, tag="ysb")
                                    for oc in range(NOC):
                                        yps = ypsp.tile([128, SC], fp32,
                                                        tag="yps")
                                        for c in range(NAC):
                                            mm(yps[:], ATN[c][:, sgl],
                                               woc[oc][:, c, :],
                                               start=(c == 0),
                                               stop=(c == NAC - 1))
                                        osl = slice(oc * SC, (oc + 1) * SC)
                                        if oc % 2 == 0:
                                            nc.vector.tensor_copy(
                                                ysb[:, osl], yps[:])
                                        else:
                                            nc.scalar.copy(ysb[:, osl], yps[:])
                                    nc.sync.dma_start(dst[sgl, :], ysb[:])
                                    if use_collective:
                                        for (r0, r1) in RS_CHUNKS:
                                            if r1 != (sg + 1) * 128:
                                                continue
                                            s0, s1 = r0 // GRP, r1 // GRP
                                            nc.gpsimd.collective_compute(
                                                "ReduceScatter",
                                                mybir.AluOpType.add,
                                                replica_groups=rgroups,
                                                ins=[y_part[r0:r1, :]],
                                                outs=[rs_out[s0:s1, :]],
                                            )
                                            nc.sync.dma_start(
                                                y_e[s0:s1, :],
                                                rs_out[s0:s1, :])

                    if debug_taps:
                        for c in range(NHC):
                            nc.sync.dma_start(dbg["dbg_qt"][hg, c], QT[c][:])
                            nc.sync.dma_start(dbg["dbg_kt"][hg, c], KT[c][:])
                        for ss in range(S // 128):
                            nc.sync.dma_start(
                                dbg["dbg_v"][hg, ss * 128:(ss + 1) * 128, :],
                                V[ss][:])

    nc.compile()
    return nc


# ---------------------------------------------------------------- host prep

def _sinusoidal_np(num_pos, dim):
    inv_freq = 1.0 / (10000.0 ** (np.arange(0, dim, 2, dtype=np.float32) / dim))
    t = np.arange(num_pos, dtype=np.float32)[:, None] * inv_freq[None, :]
    return np.cos(t).astype(np.float32), np.sin(t).astype(np.float32)  # [P, dim//2]


def _host_arrays(hs, Wq, Wk, Wv, Wo, position_ids, cfg=None, n_cores=N_CORES):
    """Build the shared + per-core input arrays (pre-tiled for DMA)."""
    import ml_dtypes
    bf16 = ml_dtypes.bfloat16

    hsT = hs.transpose(0, 2, 1).astype(bf16)                 # [B, D, S]
    # pre-tile hsT: [B][NG, 2, 128, GDC, S//2]
    hsT_t = np.ascontiguousarray(
        hsT.reshape(B, NG, GDC, 128, 2, S // 2).transpose(0, 1, 4, 3, 2, 5))

    cos_t, sin_t = _sinusoidal_np(MAX_POS, ROT)              # [P, ROT//2]
    pos = np.asarray(position_ids).astype(np.int64)          # [B, S]
    cosg = cos_t[pos]                                        # [B, S, 32]
    sing = sin_t[pos]
    cosb = np.repeat(cosg.transpose(0, 2, 1), 2, axis=1)     # [B, 64, S]
    sinb_r = np.repeat(sing.transpose(0, 2, 1), 2, axis=1)
    sgn = np.ones((ROT, 1), np.float32)
    sgn[0::2] = -1.0
    sinb = (sinb_r * sgn).astype(bf16)
    cosb = np.ascontiguousarray(cosb).astype(bf16)

    # transposed causal masks: mskT[kk, m, qq] = 0 if m*128+kk <= qq else NEG
    kk = np.arange(128)[:, None, None]
    m_ = np.arange(4)[None, :, None]
    qq = np.arange(SC)[None, None, :]
    mskT = np.where(m_ * 128 + kk <= qq, 0.0, NEG).astype(np.float32)
    mskT = np.ascontiguousarray(mskT)

    pswap = np.zeros((128, ROT), bf16)
    for f in range(ROT // 2):
        pswap[2 * f + 1, 2 * f] = 1.0
        pswap[2 * f, 2 * f + 1] = 1.0

    def tile_w(wT):       # [D, HDL] -> [NHG, NG, 128, GDC, HGW]
        return np.ascontiguousarray(
            wT.reshape(NG, GDC, 128, NHG, HGW).transpose(3, 0, 2, 1, 4))

    per_core = []
    for c in range(n_cores):
        beta, t = c // GRP, c % GRP
        csl = slice(t * HDL, (t + 1) * HDL)
        wqT = Wq[csl, :].T.astype(bf16)                      # [D, HDL]
        wkT = Wk[csl, :].T.astype(bf16)
        wvT = Wv[csl, :].T.astype(bf16)
        woT = Wo[:, csl].T.astype(bf16)                      # [HDL, D]
        wo_t = np.ascontiguousarray(
            woT.reshape(HDL // 128, 128, NOC, SC).transpose(2, 1, 0, 3))
        per_core.append(dict(
            hsT=hsT_t[beta],
            wq=tile_w(wqT), wk=tile_w(wkT), wv=tile_w(wvT), wo=wo_t,
            cosb=cosb[beta], sinb=sinb[beta],
            maskT=mskT, pswap=pswap,
        ))
    return per_core


RS_CHUNKS_HOST = [(0, 512), (512, 896), (896, 1280), (1280, 1664),
                  (1664, 1920), (1920, 2048)]


def assemble_output(outs, use_collective=True):
    """Reassemble per-core 'y' outputs into the full [B, S, D] fp32 result."""
    y = np.zeros((B, S, D), np.float32)
    for c, o in enumerate(outs):
        o = np.asarray(o, np.float32)        # [S//GRP, D] or [S, D]
        beta, t = c // GRP, c % GRP
        if use_collective:
            for (r0, r1) in RS_CHUNKS_HOST:
                L = (r1 - r0) // GRP
                y[beta, r0 + t * L:r0 + (t + 1) * L, :] = \
                    o[r0 // GRP:r0 // GRP + L]
        else:
            y[beta] += o
    return y


def _numpy_reference(hidden_states, Wq, Wk, Wv, Wo, layer_past_k, layer_past_v,
                     attention_mask, position_ids, new_key_loc, new_value_loc,
                     valid_key_indices, valid_value_indices, bucket_size):
    """Slow but general fallback (mirrors reference.py in numpy fp32)."""
    hs = np.asarray(hidden_states, np.float32)
    Bc, Sc, Dc = hs.shape
    q = (hs @ np.asarray(Wq).T).reshape(Bc, Sc, NH, HD)
    k = (hs @ np.asarray(Wk).T).reshape(Bc, Sc, NH, HD)
    v = (hs @ np.asarray(Wv).T).reshape(Bc, Sc, NH, HD)

    cos_t, sin_t = _sinusoidal_np(MAX_POS, ROT)
    pos = np.asarray(position_ids).astype(np.int64)
    c_ = cos_t[pos][:, :, None, :]      # [B,S,1,32]
    s_ = sin_t[pos][:, :, None, :]

    def rot(x):
        xr = x[..., :ROT].reshape(Bc, Sc, NH, ROT // 2, 2)
        x0, x1 = xr[..., 0], xr[..., 1]
        o0 = c_ * x0 - s_ * x1
        o1 = s_ * x0 + c_ * x1
        out = np.stack([o0, o1], axis=-1).reshape(Bc, Sc, NH, ROT)
        return np.concatenate([out, x[..., ROT:]], axis=-1)

    q, k = rot(q), rot(k)
    nk = np.asarray(layer_past_k, np.float32).copy()
    nv = np.asarray(layer_past_v, np.float32).copy()
    nk[np.asarray(new_key_loc)] = k.reshape(Bc * Sc, 1, NH, HD)
    nv[np.asarray(new_value_loc)] = v.reshape(Bc * Sc, 1, NH, HD)
    kg = nk[np.asarray(valid_key_indices)].reshape(
        Bc, bucket_size, NH, HD).transpose(0, 2, 1, 3)
    vg = nv[np.asarray(valid_value_indices)].reshape(
        Bc, bucket_size, NH, HD).transpose(0, 2, 1, 3)
    qh = q.transpose(0, 2, 1, 3)
    scores = np.einsum("bhqd,bhkd->bhqk", qh, kg)
    causal = np.tril(np.ones((MAX_POS, MAX_POS), bool))[
        bucket_size - Sc:bucket_size, :bucket_size]
    scores = np.where(causal, scores, np.float32(np.finfo(np.float32).min))
    scores = scores / np.float32(np.sqrt(HD)) + np.asarray(attention_mask, np.float32)
    scores = scores - scores.max(-1, keepdims=True)
    p = np.exp(scores)
    p = p / p.sum(-1, keepdims=True)
    attn = np.einsum("bhqk,bhkd->bhqd", p, vg)
    attn = attn.transpose(0, 2, 1, 3).reshape(Bc, Sc, Dc)
    return (attn @ np.asarray(Wo).T).astype(np.float32)


def _fast_path_ok(layer_past_k, layer_past_v, attention_mask, new_key_loc,
                  new_value_loc, valid_key_indices, valid_value_indices,
                  bucket_size, hs_shape):
    Bc, Sc, Dc = hs_shape
    if (Bc, Sc, Dc) != (B, S, D) or int(bucket_size) != S:
        return False
    ar = np.arange(Bc * Sc)
    for idx in (new_key_loc, new_value_loc, valid_key_indices, valid_value_indices):
        a = np.asarray(idx)
        if a.shape != (Bc * Sc,) or not np.array_equal(a, ar):
            return False
    if np.any(np.asarray(attention_mask) != 0):
        return False
    return True


_NC_CACHE = {}


def _get_nc(use_collective=True):
    key = ("v2", use_collective)
    if key not in _NC_CACHE:
        _NC_CACHE[key] = build_nc(use_collective=use_collective,
                                  n_cores=N_CORES)
    return _NC_CACHE[key]


def kernel(**inputs):
    hs = np.asarray(inputs["hidden_states"], np.float32)
    fast = _fast_path_ok(
        inputs["layer_past_k"], inputs["layer_past_v"], inputs["attention_mask"],
        inputs["new_key_loc"], inputs["new_value_loc"],
        inputs["valid_key_indices"], inputs["valid_value_indices"],
        inputs["bucket_size"], hs.shape)
    if not fast:
        return _numpy_reference(**inputs)

    from concourse.bass_utils import run_bass_kernel_spmd

    use_collective = os.environ.get("KERNEL_NO_COLLECTIVE", "") != "1"
    nc = _get_nc(use_collective)
    in_maps = _host_arrays(
        hs, np.asarray(inputs["Wq"], np.float32),
        np.asarray(inputs["Wk"], np.float32),
        np.asarray(inputs["Wv"], np.float32),
        np.asarray(inputs["Wo"], np.float32),
        inputs["position_ids"])
    res = run_bass_kernel_spmd(nc, in_maps, list(range(N_CORES)))
    outs = [res.results[c]["y"] for c in range(N_CORES)]
    return assemble_output(outs, use_collective)


# revision 33
# speedup vs baseline: 1.1389x; 1.1389x over previous
"""GPT-J attention (B=2, S=2048, D=4096, 16 heads x 256, partial RoPE 64) on 8 trn2 cores.

Sharding: DP x TP = 2 batches x 4 head-groups. Core c owns batch c//4 and
heads [4*(c%4), 4*(c%4)+4) (Wq/Wk/Wv column slices of 1024, Wo row slice).
Each core computes its 4 heads' attention for its batch, then the partial
out-projection [2048, 4096]; a 4-core ReduceScatter (per column chunk,
overlapped with the out-projection) sums partials and leaves each core with
a [512, col-chunk] shard; host reassembles.

Per-core kernel (all bf16 matmuls, fp32 PSUM accumulation):
  - phase A (x2 head-groups hg of 512 cols): QKV projection with d streamed
    in 4 groups of 1024; partials accumulated into SBUF bf16 (DVE).
    hsT/weights are host-pre-tiled so every DMA is 128 contiguous
    per-partition descriptors.
  - RoPE via pair-swap PE matmul + DVE muls on the first 64 rows of each
    head (cos/sin gathered per position on host).
  - attention (per head, 512-query macro tiles): scores computed
    TRANSPOSED (k on partitions, q on free) so exp'd probs PT feed PV
    directly with no PE transpose / PSUM->SBUF copy. Rowsums via a
    ones-vector matmul into psum row 0, reciprocal on DVE, broadcast to
    128 partitions on Pool (gpsimd), applied at ATN evacuation (DVE).
  - phase C: out-proj accumulates all 8 ATN chunks in one PSUM chain;
    evacuation alternates DVE/ACT to bf16; ReduceScatter issued per
    1024-column chunk so the collective overlaps the remaining compute.
"""

import os
import sys

import numpy as np

sys.path.insert(0, "/opt/trn_rl_repo")

# ---------------------------------------------------------------- constants
B = 2
S = 2048
D = 4096
NH = 16
HD = 256
ROT = 64
MAX_POS = 2048
N_CORES = 8
GRP = 4                      # replica-group size (TP groups per batch)
HPC = NH // GRP              # heads per core = 4
HDL = HPC * HD               # local head width = 1024
NHG = 2                      # head-groups per core
HGW = HDL // NHG             # head-group width = 512
NHC = HGW // 128             # chunks per head-group = 4
NG = 4                       # d-groups
DG = D // NG                 # 1024
GDC = DG // 128              # 8
SC = 512                     # s-chunk
NSC = S // SC                # 4
NOC = D // SC                # out-proj column strips = 8
NCH = 4                      # collective chunks (pairs of strips)
CW = D // NCH                # 1024 cols per chunk
SHARD = S // GRP             # 512 rows per core shard
NEG = -1.0e30


def _cfg_full():
    return dict(B=B, S=S, D=D, HPC=HPC, HD=HD, ROT=ROT)


# ---------------------------------------------------------------- bass build

def build_nc(use_collective=True, n_cores=N_CORES, mm_dtype="bfloat16",
             debug_taps=False):
    import concourse.tile as tile
    from concourse import bacc, mybir

    fp32 = mybir.dt.float32
    mdt = getattr(mybir.dt, mm_dtype)

    nc = bacc.Bacc(num_devices=n_cores)

    # per-core inputs (host-pre-tiled for contiguous per-partition DMA)
    hsT_e = nc.declare_dram_parameter("hsT", [NG, 2, 128, GDC, S // 2], mdt,
                                      isOutput=False)
    wq_e = nc.declare_dram_parameter("wq", [NHG, NG, 128, GDC, HGW], mdt,
                                     isOutput=False)
    wk_e = nc.declare_dram_parameter("wk", [NHG, NG, 128, GDC, HGW], mdt,
                                     isOutput=False)
    wv_e = nc.declare_dram_parameter("wv", [NHG, NG, 128, GDC, HGW], mdt,
                                     isOutput=False)
    wo_e = nc.declare_dram_parameter("wo", [NOC, 128, HDL // 128, SC], mdt,
                                     isOutput=False)
    cos_e = nc.declare_dram_parameter("cosb", [ROT, S], mdt, isOutput=False)
    sin_e = nc.declare_dram_parameter("sinb", [ROT, S], mdt, isOutput=False)
    mskT_e = nc.declare_dram_parameter("maskT", [128, 4, SC], fp32,
                                       isOutput=False)
    psw_e = nc.declare_dram_parameter("pswap", [128, ROT], mdt, isOutput=False)

    # collective row-chunks: issued after the sg-block that completes them.
    # finer chunks near the end shrink the exposed tail.
    RS_CHUNKS = [(0, 512), (512, 1024), (1024, 1536), (1536, 1920),
                 (1920, 2048)]
    if use_collective:
        y_e = nc.declare_dram_parameter("y", [S // GRP, D], mdt,
                                        isOutput=True)
        y_part = nc.dram_tensor("y_part", [S, D], mdt)
        rs_out = nc.dram_tensor("rs_out", [S // GRP, D], mdt)
        cc_warm_in = nc.dram_tensor("cc_warm_in", [GRP * 128], mdt)
        cc_warm_out = nc.dram_tensor("cc_warm_out", [128], mdt)
        rgroups = [[0, 1, 2, 3], [4, 5, 6, 7]]
    else:
        y_e = nc.declare_dram_parameter("y", [S, D], mdt, isOutput=True)

    if debug_taps:
        dbg = {}
        for nm, shp in [("dbg_qt", [NHG, NHC, 128, S]),
                        ("dbg_kt", [NHG, NHC, 128, S]),
                        ("dbg_v", [NHG, S, HGW]),
                        ("dbg_atn", [NHG * NHC, 128, S])]:
            dbg[nm] = nc.declare_dram_parameter(nm, shp, mdt, isOutput=True)

    def mm(ps, lhsT, rhs, start, stop):
        nc.tensor.matmul(ps, lhsT, rhs, start=start, stop=stop)

    with tile.TileContext(nc) as tc:
        with tc.tile_pool(name="const", bufs=1) as constp:
            mskT = constp.tile([128, 4, SC], fp32)
            nc.sync.dma_start(mskT[:], mskT_e[:])
            pswap = constp.tile([128, ROT], mdt)
            nc.sync.dma_start(pswap[:], psw_e[:])
            onesq = constp.tile([128, 128], mdt)
            nc.vector.memset(onesq[:], 1.0)
            cosb = constp.tile([ROT, S], mdt)
            sinb = constp.tile([ROT, S], mdt)
            nc.sync.dma_start(cosb[:], cos_e[:])
            nc.sync.dma_start(sinb[:], sin_e[:])
            if use_collective:
                # tiny warmup collective: absorbs mesh-algo init cost while
                # phase A runs (input zero-filled first — garbage bits can
                # be NaN and upset the CC cores)
                warmz = constp.tile([128, GRP], mdt)
                nc.vector.memset(warmz[:], 0.0)
                nc.sync.dma_start(
                    cc_warm_in[:].rearrange("(p g) -> p g", p=128), warmz[:])
                nc.gpsimd.collective_compute(
                    "ReduceScatter", mybir.AluOpType.add,
                    replica_groups=rgroups,
                    ins=[cc_warm_in[:]], outs=[cc_warm_out[:]])

            with (
                tc.tile_pool(name="qkv", bufs=1) as qkvp,
                tc.tile_pool(name="atn", bufs=1) as atnp,
            ):
                ATN = [atnp.tile([128, S], mdt, tag=f"ATN{c}", name=f"ATN{c}")
                       for c in range(NHG * NHC)]
                for hg in range(NHG):
                    QT = [qkvp.tile([128, S], mdt, tag=f"QT{c}", name=f"QT{c}")
                          for c in range(NHC)]
                    KT = [qkvp.tile([128, S], mdt, tag=f"KT{c}", name=f"KT{c}")
                          for c in range(NHC)]
                    V = [qkvp.tile([128, HGW], mdt, tag=f"V{ss}", name=f"V{ss}")
                         for ss in range(S // 128)]

                    # ---------------- phase A: QKV projection ----------------
                    with (
                        tc.tile_pool(name="wts", bufs=1) as wp,
                        tc.tile_pool(name="hst", bufs=2) as hp,
                        tc.tile_pool(name="pjps", bufs=1, space="PSUM") as pjps,
                    ):
                        for g in range(NG):
                            wq = wp.tile([128, GDC, HGW], mdt, tag="wq")
                            wk = wp.tile([128, GDC, HGW], mdt, tag="wk")
                            wv = wp.tile([128, GDC, HGW], mdt, tag="wv")
                            nc.sync.dma_start(wq[:], wq_e[hg, g])
                            nc.sync.dma_start(wk[:], wk_e[hg, g])
                            nc.sync.dma_start(wv[:], wv_e[hg, g])
                            for sh in range(2):
                                hst = hp.tile([128, GDC, S // 2], mdt, tag="hst")
                                nc.sync.dma_start(hst[:], hsT_e[g, sh])
                                for sc2 in range(2):
                                    sc = sh * 2 + sc2
                                    ssl = slice(sc * SC, (sc + 1) * SC)
                                    hsl = slice(sc2 * SC, (sc2 + 1) * SC)
                                    psq = [pjps.tile([128, SC], fp32,
                                                     tag=f"psq{h}", name=f"psq{h}")
                                           for h in range(NHC)]
                                    psk = [pjps.tile([128, SC], fp32,
                                                     tag=f"psk{h}", name=f"psk{h}")
                                           for h in range(NHC)]
                                    for dc in range(GDC):
                                        for h in range(NHC):
                                            osl = slice(h * 128, (h + 1) * 128)
                                            mm(psq[h][:], wq[:, dc, osl],
                                               hst[:, dc, hsl],
                                               start=(dc == 0), stop=(dc == GDC - 1))
                                            mm(psk[h][:], wk[:, dc, osl],
                                               hst[:, dc, hsl],
                                               start=(dc == 0), stop=(dc == GDC - 1))
                                    for h in range(NHC):
                                        if g == 0:
                                            nc.vector.tensor_copy(QT[h][:, ssl], psq[h][:])
                                            nc.vector.tensor_copy(KT[h][:, ssl], psk[h][:])
                                        else:
                                            nc.vector.tensor_add(
                                                QT[h][:, ssl], QT[h][:, ssl], psq[h][:])
                                            nc.vector.tensor_add(
                                                KT[h][:, ssl], KT[h][:, ssl], psk[h][:])
                                    psv = [pjps.tile([128, HGW], fp32,
                                                     tag=f"psq{ss}", name=f"psv{ss}")
                                           for ss in range(SC // 128)]
                                    for dc in range(GDC):
                                        for ss in range(SC // 128):
                                            ssub = slice(sc2 * SC + ss * 128,
                                                         sc2 * SC + (ss + 1) * 128)
                                            mm(psv[ss][:], hst[:, dc, ssub],
                                               wv[:, dc, :],
                                               start=(dc == 0), stop=(dc == GDC - 1))
                                    for ss in range(SC // 128):
                                        vi = sc * (SC // 128) + ss
                                        if g == 0:
                                            nc.scalar.copy(V[vi][:], psv[ss][:])
                                        else:
                                            nc.vector.tensor_add(
                                                V[vi][:], V[vi][:], psv[ss][:])

                    # ---------------- RoPE on QT/KT rot rows ----------------
                    with (
                        tc.tile_pool(name="rope", bufs=4) as ropep,
                        tc.tile_pool(name="rops", bufs=2, space="PSUM") as ropsp,
                    ):
                        for t in (QT, KT):
                            for hch in range(0, NHC, HD // 128):
                                for sc in range(NSC):
                                    ssl = slice(sc * SC, (sc + 1) * SC)
                                    sw = ropsp.tile([ROT, SC], fp32, tag="sw")
                                    mm(sw[:], pswap[:, :], t[hch][:, ssl],
                                       start=True, stop=True)
                                    t1 = ropep.tile([ROT, SC], mdt, tag="t1")
                                    t2 = ropep.tile([ROT, SC], mdt, tag="t2")
                                    nc.vector.tensor_tensor(
                                        t1[:], sw[:], sinb[:, ssl],
                                        op=mybir.AluOpType.mult)
                                    nc.vector.tensor_tensor(
                                        t2[:], t[hch][0:ROT, ssl], cosb[:, ssl],
                                        op=mybir.AluOpType.mult)
                                    nc.vector.tensor_add(t[hch][0:ROT, ssl],
                                                         t1[:], t2[:])

                    # ---------------- phase B: attention (transposed) --------
                    def attn_qm(pools, hg, qm):
                        ptp, rcpp, scps, atps, rsps = pools
                        qsl = slice(qm * SC, (qm + 1) * SC)
                        nkc = (qm + 1) * (SC // 128)
                        for h in range(HPC // NHG):
                            c0 = h * (HD // 128)
                            atn_ps = [atps.tile([128, SC], fp32,
                                                tag=f"atn{hh}", name=f"atn{hh}")
                                      for hh in range(HD // 128)]
                            rs_ps = rsps.tile([128, SC], fp32, tag="rs")
                            for kc in range(nkc):
                                kcl = slice(kc * 128, (kc + 1) * 128)
                                sT = scps.tile([128, SC], fp32, tag="sT")
                                mm(sT[:], KT[c0][:, kcl], QT[c0][:, qsl],
                                   start=True, stop=False)
                                mm(sT[:], KT[c0 + 1][:, kcl], QT[c0 + 1][:, qsl],
                                   start=False, stop=True)
                                if kc >= nkc - 4:
                                    nc.vector.tensor_add(
                                        sT[:], sT[:], mskT[:, kc - (nkc - 4), :])
                                PT = ptp.tile([128, SC], mdt, tag="PT")
                                nc.scalar.activation(
                                    PT[:], sT[:],
                                    mybir.ActivationFunctionType.Exp,
                                    bias=0.0, scale=1.0 / 16.0)
                                for hh in range(HD // 128):
                                    vsl = slice(h * HD + hh * 128,
                                                h * HD + (hh + 1) * 128)
                                    mm(atn_ps[hh][:], V[kc][:, vsl], PT[:],
                                       start=(kc == 0), stop=(kc == nkc - 1))
                                mm(rs_ps[:], onesq[:, :], PT[:],
                                   start=(kc == 0), stop=(kc == nkc - 1))
                            rcp = rcpp.tile([128, SC], fp32, tag="rcp")
                            # rowsums are in [1, ~2e7]: safe for the approx
                            nc.vector.reciprocal_approx_fast(rcp[:], rs_ps[:])
                            for hh in range(HD // 128):
                                nc.vector.tensor_tensor(
                                    ATN[hg * NHC + c0 + hh][:, qsl],
                                    atn_ps[hh][:], rcp[:],
                                    op=mybir.AluOpType.mult)

                    NAC = NHG * NHC      # 8 ATN chunks
                    if hg == 0:
                        with (
                            tc.tile_pool(name="ptsb", bufs=3) as ptp,
                            tc.tile_pool(name="rcp", bufs=2) as rcpp,
                            tc.tile_pool(name="scps", bufs=2, space="PSUM") as scps,
                            tc.tile_pool(name="atps", bufs=1, space="PSUM") as atps,
                            tc.tile_pool(name="rsps", bufs=1, space="PSUM") as rsps,
                        ):
                            for qm in range(NSC):
                                attn_qm((ptp, rcpp, scps, atps, rsps), 0, qm)
                    else:
                        # fused: attention(hg1, qm) -> out-proj rows 4qm..4qm+3
                        # -> ReduceScatter chunk qm (overlaps next qm compute)
                        with (
                            tc.tile_pool(name="ptsb", bufs=3) as ptp,
                            tc.tile_pool(name="rcp", bufs=2) as rcpp,
                            tc.tile_pool(name="wo", bufs=1) as wop,
                            tc.tile_pool(name="ysb", bufs=3) as ysbp,
                            tc.tile_pool(name="scps", bufs=2, space="PSUM") as scps,
                            tc.tile_pool(name="atps", bufs=1, space="PSUM") as atps,
                            tc.tile_pool(name="rsps", bufs=1, space="PSUM") as rsps,
                            tc.tile_pool(name="yps", bufs=3, space="PSUM") as ypsp,
                        ):
                            woc = [wop.tile([128, NAC, SC], mdt, tag=f"woc{oc}",
                                            name=f"woc{oc}") for oc in range(NOC)]
                            for oc in range(NOC):
                                nc.sync.dma_start(woc[oc][:], wo_e[oc])
                            dst = y_part if use_collective else y_e
                            for qm in range(NSC):
                                attn_qm((ptp, rcpp, scps, atps, rsps), 1, qm)
                                for sg in range(qm * 4, qm * 4 + 4):
                                    sgl = slice(sg * 128, (sg + 1) * 128)
                                    ysb = ysbp.tile([128, D], mdt, tag="ysb")
                                    for oc in range(NOC):
                                        yps = ypsp.tile([128, SC], fp32,
                                                        tag="yps")
                                        for c in range(NAC):
                                            mm(yps[:], ATN[c][:, sgl],
                                               woc[oc][:, c, :],
                                               start=(c == 0),
                                               stop=(c == NAC - 1))
                                        osl = slice(oc * SC, (oc + 1) * SC)
                                        if oc % 2 == 0:
                                            nc.vector.tensor_copy(
                                                ysb[:, osl], yps[:])
                                        else:
                                            nc.scalar.copy(ysb[:, osl], yps[:])
                                    nc.sync.dma_start(dst[sgl, :], ysb[:])
                                    if use_collective:
                                        for (r0, r1) in RS_CHUNKS:
                                            if r1 != (sg + 1) * 128:
                                                continue
                                            s0, s1 = r0 // GRP, r1 // GRP
                                            nc.gpsimd.collective_compute(
                                                "ReduceScatter",
                                                mybir.AluOpType.add,
                                                replica_groups=rgroups,
                                                ins=[y_part[r0:r1, :]],
                                                outs=[rs_out[s0:s1, :]],
                                            )
                                            nc.sync.dma_start(
                                                y_e[s0:s1, :],
                                                rs_out[s0:s1, :])

                    if debug_taps:
                        for c in range(NHC):
                            nc.sync.dma_start(dbg["dbg_qt"][hg, c], QT[c][:])
                            nc.sync.dma_start(dbg["dbg_kt"][hg, c], KT[c][:])
                        for ss in range(S // 128):
                            nc.sync.dma_start(
                                dbg["dbg_v"][hg, ss * 128:(ss + 1) * 128, :],
                                V[ss][:])

    nc.compile()
    return nc


# ---------------------------------------------------------------- host prep

def _sinusoidal_np(num_pos, dim):
    inv_freq = 1.0 / (10000.0 ** (np.arange(0, dim, 2, dtype=np.float32) / dim))
    t = np.arange(num_pos, dtype=np.float32)[:, None] * inv_freq[None, :]
    return np.cos(t).astype(np.float32), np.sin(t).astype(np.float32)  # [P, dim//2]


def _host_arrays(hs, Wq, Wk, Wv, Wo, position_ids, cfg=None, n_cores=N_CORES):
    """Build the shared + per-core input arrays (pre-tiled for DMA)."""
    import ml_dtypes
    bf16 = ml_dtypes.bfloat16

    hsT = hs.transpose(0, 2, 1).astype(bf16)                 # [B, D, S]
    # pre-tile hsT: [B][NG, 2, 128, GDC, S//2]
    hsT_t = np.ascontiguousarray(
        hsT.reshape(B, NG, GDC, 128, 2, S // 2).transpose(0, 1, 4, 3, 2, 5))

    cos_t, sin_t = _sinusoidal_np(MAX_POS, ROT)              # [P, ROT//2]
    pos = np.asarray(position_ids).astype(np.int64)          # [B, S]
    cosg = cos_t[pos]                                        # [B, S, 32]
    sing = sin_t[pos]
    cosb = np.repeat(cosg.transpose(0, 2, 1), 2, axis=1)     # [B, 64, S]
    sinb_r = np.repeat(sing.transpose(0, 2, 1), 2, axis=1)
    sgn = np.ones((ROT, 1), np.float32)
    sgn[0::2] = -1.0
    sinb = (sinb_r * sgn).astype(bf16)
    cosb = np.ascontiguousarray(cosb).astype(bf16)

    # transposed causal masks: mskT[kk, m, qq] = 0 if m*128+kk <= qq else NEG
    kk = np.arange(128)[:, None, None]
    m_ = np.arange(4)[None, :, None]
    qq = np.arange(SC)[None, None, :]
    mskT = np.where(m_ * 128 + kk <= qq, 0.0, NEG).astype(np.float32)
    mskT = np.ascontiguousarray(mskT)

    pswap = np.zeros((128, ROT), bf16)
    for f in range(ROT // 2):
        pswap[2 * f + 1, 2 * f] = 1.0
        pswap[2 * f, 2 * f + 1] = 1.0

    def tile_w(wT):       # [D, HDL] -> [NHG, NG, 128, GDC, HGW]
        return np.ascontiguousarray(
            wT.reshape(NG, GDC, 128, NHG, HGW).transpose(3, 0, 2, 1, 4))

    per_core = []
    for c in range(n_cores):
        beta, t = c // GRP, c % GRP
        csl = slice(t * HDL, (t + 1) * HDL)
        wqT = Wq[csl, :].T.astype(bf16)                      # [D, HDL]
        wkT = Wk[csl, :].T.astype(bf16)
        wvT = Wv[csl, :].T.astype(bf16)
        woT = Wo[:, csl].T.astype(bf16)                      # [HDL, D]
        wo_t = np.ascontiguousarray(
            woT.reshape(HDL // 128, 128, NOC, SC).transpose(2, 1, 0, 3))
        per_core.append(dict(
            hsT=hsT_t[beta],
            wq=tile_w(wqT), wk=tile_w(wkT), wv=tile_w(wvT), wo=wo_t,
            cosb=cosb[beta], sinb=sinb[beta],
            maskT=mskT, pswap=pswap,
        ))
    return per_core


RS_CHUNKS_HOST = [(0, 512), (512, 1024), (1024, 1536), (1536, 1920),
                  (1920, 2048)]


def assemble_output(outs, use_collective=True):
    """Reassemble per-core 'y' outputs into the full [B, S, D] fp32 result."""
    y = np.zeros((B, S, D), np.float32)
    for c, o in enumerate(outs):
        o = np.asarray(o, np.float32)        # [S//GRP, D] or [S, D]
        beta, t = c // GRP, c % GRP
        if use_collective:
            for (r0, r1) in RS_CHUNKS_HOST:
                L = (r1 - r0) // GRP
                y[beta, r0 + t * L:r0 + (t + 1) * L, :] = \
                    o[r0 // GRP:r0 // GRP + L]
        else:
            y[beta] += o
    return y


def _numpy_reference(hidden_states, Wq, Wk, Wv, Wo, layer_past_k, layer_past_v,
                     attention_mask, position_ids, new_key_loc, new_value_loc,
                     valid_key_indices, valid_value_indices, bucket_size):
    """Slow but general fallback (mirrors reference.py in numpy fp32)."""
    hs = np.asarray(hidden_states, np.float32)
    Bc, Sc, Dc = hs.shape
    q = (hs @ np.asarray(Wq).T).reshape(Bc, Sc, NH, HD)
    k = (hs @ np.asarray(Wk).T).reshape(Bc, Sc, NH, HD)
    v = (hs @ np.asarray(Wv).T).reshape(Bc, Sc, NH, HD)

    cos_t, sin_t = _sinusoidal_np(MAX_POS, ROT)
    pos = np.asarray(position_ids).astype(np.int64)
    c_ = cos_t[pos][:, :, None, :]      # [B,S,1,32]
    s_ = sin_t[pos][:, :, None, :]

    def rot(x):
        xr = x[..., :ROT].reshape(Bc, Sc, NH, ROT // 2, 2)
        x0, x1 = xr[..., 0], xr[..., 1]
        o0 = c_ * x0 - s_ * x1
        o1 = s_ * x0 + c_ * x1
        out = np.stack([o0, o1], axis=-1).reshape(Bc, Sc, NH, ROT)
        return np.concatenate([out, x[..., ROT:]], axis=-1)

    q, k = rot(q), rot(k)
    nk = np.asarray(layer_past_k, np.float32).copy()
    nv = np.asarray(layer_past_v, np.float32).copy()
    nk[np.asarray(new_key_loc)] = k.reshape(Bc * Sc, 1, NH, HD)
    nv[np.asarray(new_value_loc)] = v.reshape(Bc * Sc, 1, NH, HD)
    kg = nk[np.asarray(valid_key_indices)].reshape(
        Bc, bucket_size, NH, HD).transpose(0, 2, 1, 3)
    vg = nv[np.asarray(valid_value_indices)].reshape(
        Bc, bucket_size, NH, HD).transpose(0, 2, 1, 3)
    qh = q.transpose(0, 2, 1, 3)
    scores = np.einsum("bhqd,bhkd->bhqk", qh, kg)
    causal = np.tril(np.ones((MAX_POS, MAX_POS), bool))[
        bucket_size - Sc:bucket_size, :bucket_size]
    scores = np.where(causal, scores, np.float32(np.finfo(np.float32).min))
    scores = scores / np.float32(np.sqrt(HD)) + np.asarray(attention_mask, np.float32)
    scores = scores - scores.max(-1, keepdims=True)
    p = np.exp(scores)
    p = p / p.sum(-1, keepdims=True)
    attn = np.einsum("bhqk,bhkd->bhqd", p, vg)
    attn = attn.transpose(0, 2, 1, 3).reshape(Bc, Sc, Dc)
    return (attn @ np.asarray(Wo).T).astype(np.float32)


def _fast_path_ok(layer_past_k, layer_past_v, attention_mask, new_key_loc,
                  new_value_loc, valid_key_indices, valid_value_indices,
                  bucket_size, hs_shape):
    Bc, Sc, Dc = hs_shape
    if (Bc, Sc, Dc) != (B, S, D) or int(bucket_size) != S:
        return False
    ar = np.arange(Bc * Sc)
    for idx in (new_key_loc, new_value_loc, valid_key_indices, valid_value_indices):
        a = np.asarray(idx)
        if a.shape != (Bc * Sc,) or not np.array_equal(a, ar):
            return False
    if np.any(np.asarray(attention_mask) != 0):
        return False
    return True


_NC_CACHE = {}


def _get_nc(use_collective=True):
    key = ("v2", use_collective)
    if key not in _NC_CACHE:
        _NC_CACHE[key] = build_nc(use_collective=use_collective,
                                  n_cores=N_CORES)
    return _NC_CACHE[key]


def kernel(**inputs):
    hs = np.asarray(inputs["hidden_states"], np.float32)
    fast = _fast_path_ok(
        inputs["layer_past_k"], inputs["layer_past_v"], inputs["attention_mask"],
        inputs["new_key_loc"], inputs["new_value_loc"],
        inputs["valid_key_indices"], inputs["valid_value_indices"],
        inputs["bucket_size"], hs.shape)
    if not fast:
        return _numpy_reference(**inputs)

    from concourse.bass_utils import run_bass_kernel_spmd

    use_collective = os.environ.get("KERNEL_NO_COLLECTIVE", "") != "1"
    nc = _get_nc(use_collective)
    in_maps = _host_arrays(
        hs, np.asarray(inputs["Wq"], np.float32),
        np.asarray(inputs["Wk"], np.float32),
        np.asarray(inputs["Wv"], np.float32),
        np.asarray(inputs["Wo"], np.float32),
        inputs["position_ids"])
    res = run_bass_kernel_spmd(nc, in_maps, list(range(N_CORES)))
    outs = [res.results[c]["y"] for c in range(N_CORES)]
    return assemble_output(outs, use_collective)


# revision 36
# speedup vs baseline: 1.1576x; 1.0164x over previous
"""GPT-J attention (B=2, S=2048, D=4096, 16 heads x 256, partial RoPE 64) on 8 trn2 cores.

Sharding: DP x TP = 2 batches x 4 head-groups. Core c owns batch c//4 and
heads [4*(c%4), 4*(c%4)+4) (Wq/Wk/Wv column slices of 1024, Wo row slice).
Each core computes its 4 heads' attention for its batch, then the partial
out-projection [2048, 4096]; a 4-core ReduceScatter (per column chunk,
overlapped with the out-projection) sums partials and leaves each core with
a [512, col-chunk] shard; host reassembles.

Per-core kernel (all bf16 matmuls, fp32 PSUM accumulation):
  - phase A (x2 head-groups hg of 512 cols): QKV projection with d streamed
    in 4 groups of 1024; partials accumulated into SBUF bf16 (DVE).
    hsT/weights are host-pre-tiled so every DMA is 128 contiguous
    per-partition descriptors.
  - RoPE via pair-swap PE matmul + DVE muls on the first 64 rows of each
    head (cos/sin gathered per position on host).
  - attention (per head, 512-query macro tiles): scores computed
    TRANSPOSED (k on partitions, q on free) so exp'd probs PT feed PV
    directly with no PE transpose / PSUM->SBUF copy. Rowsums via a
    ones-vector matmul into psum row 0, reciprocal on DVE, broadcast to
    128 partitions on Pool (gpsimd), applied at ATN evacuation (DVE).
  - phase C: out-proj accumulates all 8 ATN chunks in one PSUM chain;
    evacuation alternates DVE/ACT to bf16; ReduceScatter issued per
    1024-column chunk so the collective overlaps the remaining compute.
"""

import os
import sys

import numpy as np

sys.path.insert(0, "/opt/trn_rl_repo")

# ---------------------------------------------------------------- constants
B = 2
S = 2048
D = 4096
NH = 16
HD = 256
ROT = 64
MAX_POS = 2048
N_CORES = 8
GRP = 4                      # replica-group size (TP groups per batch)
HPC = NH // GRP              # heads per core = 4
HDL = HPC * HD               # local head width = 1024
NHG = 2                      # head-groups per core
HGW = HDL // NHG             # head-group width = 512
NHC = HGW // 128             # chunks per head-group = 4
NG = 4                       # d-groups
DG = D // NG                 # 1024
GDC = DG // 128              # 8
SC = 512                     # s-chunk
NSC = S // SC                # 4
NOC = D // SC                # out-proj column strips = 8
NCH = 4                      # collective chunks (pairs of strips)
CW = D // NCH                # 1024 cols per chunk
SHARD = S // GRP             # 512 rows per core shard
NEG = -1.0e30


def _cfg_full():
    return dict(B=B, S=S, D=D, HPC=HPC, HD=HD, ROT=ROT)


# ---------------------------------------------------------------- bass build

def build_nc(use_collective=True, n_cores=N_CORES, mm_dtype="bfloat16",
             debug_taps=False):
    import concourse.tile as tile
    from concourse import bacc, mybir

    fp32 = mybir.dt.float32
    mdt = getattr(mybir.dt, mm_dtype)

    nc = bacc.Bacc(num_devices=n_cores)

    # per-core inputs (host-pre-tiled for contiguous per-partition DMA)
    hsT_e = nc.declare_dram_parameter("hsT", [NG, 2, 128, GDC, S // 2], mdt,
                                      isOutput=False)
    wq_e = nc.declare_dram_parameter("wq", [NHG, NG, 128, GDC, HGW], mdt,
                                     isOutput=False)
    wk_e = nc.declare_dram_parameter("wk", [NHG, NG, 128, GDC, HGW], mdt,
                                     isOutput=False)
    wv_e = nc.declare_dram_parameter("wv", [NHG, NG, 128, GDC, HGW], mdt,
                                     isOutput=False)
    wo_e = nc.declare_dram_parameter("wo", [NOC, 128, HDL // 128, SC], mdt,
                                     isOutput=False)
    cos_e = nc.declare_dram_parameter("cosb", [ROT, S], mdt, isOutput=False)
    sin_e = nc.declare_dram_parameter("sinb", [ROT, S], mdt, isOutput=False)
    mskT_e = nc.declare_dram_parameter("maskT", [128, 4, SC], fp32,
                                       isOutput=False)
    psw_e = nc.declare_dram_parameter("pswap", [128, ROT], mdt, isOutput=False)

    # collective row-chunks: issued after the sg-block that completes them.
    # finer chunks near the end shrink the exposed tail.
    RS_CHUNKS = [(0, 512), (512, 1024), (1024, 1536), (1536, 1792),
                 (1792, 1920), (1920, 2048)]
    if use_collective:
        y_e = nc.declare_dram_parameter("y", [S // GRP, D], mdt,
                                        isOutput=True)
        y_part = nc.dram_tensor("y_part", [S, D], mdt)
        rs_out = nc.dram_tensor("rs_out", [S // GRP, D], mdt)
        cc_warm_in = nc.dram_tensor("cc_warm_in", [GRP * 128], mdt)
        cc_warm_out = nc.dram_tensor("cc_warm_out", [128], mdt)
        rgroups = [[0, 1, 2, 3], [4, 5, 6, 7]]
    else:
        y_e = nc.declare_dram_parameter("y", [S, D], mdt, isOutput=True)

    if debug_taps:
        dbg = {}
        for nm, shp in [("dbg_qt", [NHG, NHC, 128, S]),
                        ("dbg_kt", [NHG, NHC, 128, S]),
                        ("dbg_v", [NHG, S, HGW]),
                        ("dbg_atn", [NHG * NHC, 128, S])]:
            dbg[nm] = nc.declare_dram_parameter(nm, shp, mdt, isOutput=True)

    def mm(ps, lhsT, rhs, start, stop):
        nc.tensor.matmul(ps, lhsT, rhs, start=start, stop=stop)

    with tile.TileContext(nc) as tc:
        with tc.tile_pool(name="const", bufs=1) as constp:
            mskT = constp.tile([128, 4, SC], fp32)
            nc.sync.dma_start(mskT[:], mskT_e[:])
            pswap = constp.tile([128, ROT], mdt)
            nc.sync.dma_start(pswap[:], psw_e[:])
            onesq = constp.tile([128, 128], mdt)
            nc.vector.memset(onesq[:], 1.0)
            cosb = constp.tile([ROT, S], mdt)
            sinb = constp.tile([ROT, S], mdt)
            nc.sync.dma_start(cosb[:], cos_e[:])
            nc.sync.dma_start(sinb[:], sin_e[:])
            if use_collective:
                # tiny warmup collective: absorbs mesh-algo init cost while
                # phase A runs (input zero-filled first — garbage bits can
                # be NaN and upset the CC cores)
                warmz = constp.tile([128, GRP], mdt)
                nc.vector.memset(warmz[:], 0.0)
                nc.sync.dma_start(
                    cc_warm_in[:].rearrange("(p g) -> p g", p=128), warmz[:])
                nc.gpsimd.collective_compute(
                    "ReduceScatter", mybir.AluOpType.add,
                    replica_groups=rgroups,
                    ins=[cc_warm_in[:]], outs=[cc_warm_out[:]])

            with (
                tc.tile_pool(name="qkv", bufs=1) as qkvp,
                tc.tile_pool(name="atn", bufs=1) as atnp,
            ):
                ATN = [atnp.tile([128, S], mdt, tag=f"ATN{c}", name=f"ATN{c}")
                       for c in range(NHG * NHC)]
                for hg in range(NHG):
                    QT = [qkvp.tile([128, S], mdt, tag=f"QT{c}", name=f"QT{c}")
                          for c in range(NHC)]
                    KT = [qkvp.tile([128, S], mdt, tag=f"KT{c}", name=f"KT{c}")
                          for c in range(NHC)]
                    V = [qkvp.tile([128, HGW], mdt, tag=f"V{ss}", name=f"V{ss}")
                         for ss in range(S // 128)]

                    # ---------------- phase A: QKV projection ----------------
                    with (
                        tc.tile_pool(name="wts", bufs=1) as wp,
                        tc.tile_pool(name="hst", bufs=2) as hp,
                        tc.tile_pool(name="pjps", bufs=1, space="PSUM") as pjps,
                    ):
                        for g in range(NG):
                            wq = wp.tile([128, GDC, HGW], mdt, tag="wq")
                            wk = wp.tile([128, GDC, HGW], mdt, tag="wk")
                            wv = wp.tile([128, GDC, HGW], mdt, tag="wv")
                            nc.sync.dma_start(wq[:], wq_e[hg, g])
                            nc.sync.dma_start(wk[:], wk_e[hg, g])
                            nc.sync.dma_start(wv[:], wv_e[hg, g])
                            for sh in range(2):
                                hst = hp.tile([128, GDC, S // 2], mdt, tag="hst")
                                nc.sync.dma_start(hst[:], hsT_e[g, sh])
                                for sc2 in range(2):
                                    sc = sh * 2 + sc2
                                    ssl = slice(sc * SC, (sc + 1) * SC)
                                    hsl = slice(sc2 * SC, (sc2 + 1) * SC)
                                    psq = [pjps.tile([128, SC], fp32,
                                                     tag=f"psq{h}", name=f"psq{h}")
                                           for h in range(NHC)]
                                    psk = [pjps.tile([128, SC], fp32,
                                                     tag=f"psk{h}", name=f"psk{h}")
                                           for h in range(NHC)]
                                    for dc in range(GDC):
                                        for h in range(NHC):
                                            osl = slice(h * 128, (h + 1) * 128)
                                            mm(psq[h][:], wq[:, dc, osl],
                                               hst[:, dc, hsl],
                                               start=(dc == 0), stop=(dc == GDC - 1))
                                            mm(psk[h][:], wk[:, dc, osl],
                                               hst[:, dc, hsl],
                                               start=(dc == 0), stop=(dc == GDC - 1))
                                    for h in range(NHC):
                                        if g == 0:
                                            nc.vector.tensor_copy(QT[h][:, ssl], psq[h][:])
                                            nc.vector.tensor_copy(KT[h][:, ssl], psk[h][:])
                                        else:
                                            nc.vector.tensor_add(
                                                QT[h][:, ssl], QT[h][:, ssl], psq[h][:])
                                            nc.vector.tensor_add(
                                                KT[h][:, ssl], KT[h][:, ssl], psk[h][:])
                                    psv = [pjps.tile([128, HGW], fp32,
                                                     tag=f"psq{ss}", name=f"psv{ss}")
                                           for ss in range(SC // 128)]
                                    for dc in range(GDC):
                                        for ss in range(SC // 128):
                                            ssub = slice(sc2 * SC + ss * 128,
                                                         sc2 * SC + (ss + 1) * 128)
                                            mm(psv[ss][:], hst[:, dc, ssub],
                                               wv[:, dc, :],
                                               start=(dc == 0), stop=(dc == GDC - 1))
                                    for ss in range(SC // 128):
                                        vi = sc * (SC // 128) + ss
                                        if g == 0:
                                            nc.scalar.copy(V[vi][:], psv[ss][:])
                                        else:
                                            nc.vector.tensor_add(
                                                V[vi][:], V[vi][:], psv[ss][:])

                    # ---------------- RoPE on QT/KT rot rows ----------------
                    with (
                        tc.tile_pool(name="rope", bufs=4) as ropep,
                        tc.tile_pool(name="rops", bufs=2, space="PSUM") as ropsp,
                    ):
                        for t in (QT, KT):
                            for hch in range(0, NHC, HD // 128):
                                for sc in range(NSC):
                                    ssl = slice(sc * SC, (sc + 1) * SC)
                                    sw = ropsp.tile([ROT, SC], fp32, tag="sw")
                                    mm(sw[:], pswap[:, :], t[hch][:, ssl],
                                       start=True, stop=True)
                                    t1 = ropep.tile([ROT, SC], mdt, tag="t1")
                                    t2 = ropep.tile([ROT, SC], mdt, tag="t2")
                                    nc.vector.tensor_tensor(
                                        t1[:], sw[:], sinb[:, ssl],
                                        op=mybir.AluOpType.mult)
                                    nc.vector.tensor_tensor(
                                        t2[:], t[hch][0:ROT, ssl], cosb[:, ssl],
                                        op=mybir.AluOpType.mult)
                                    nc.vector.tensor_add(t[hch][0:ROT, ssl],
                                                         t1[:], t2[:])

                    # ---------------- phase B: attention (transposed) --------
                    def attn_qm(pools, hg, qm):
                        ptp, rcpp, scps, atps, rsps = pools
                        qsl = slice(qm * SC, (qm + 1) * SC)
                        nkc = (qm + 1) * (SC // 128)
                        for h in range(HPC // NHG):
                            c0 = h * (HD // 128)
                            atn_ps = [atps.tile([128, SC], fp32,
                                                tag=f"atn{hh}", name=f"atn{hh}")
                                      for hh in range(HD // 128)]
                            rs_ps = rsps.tile([128, SC], fp32, tag="rs")
                            for kc in range(nkc):
                                kcl = slice(kc * 128, (kc + 1) * 128)
                                sT = scps.tile([128, SC], fp32, tag="sT")
                                mm(sT[:], KT[c0][:, kcl], QT[c0][:, qsl],
                                   start=True, stop=False)
                                mm(sT[:], KT[c0 + 1][:, kcl], QT[c0 + 1][:, qsl],
                                   start=False, stop=True)
                                if kc >= nkc - 4:
                                    nc.vector.tensor_add(
                                        sT[:], sT[:], mskT[:, kc - (nkc - 4), :])
                                PT = ptp.tile([128, SC], mdt, tag="PT")
                                nc.scalar.activation(
                                    PT[:], sT[:],
                                    mybir.ActivationFunctionType.Exp,
                                    bias=0.0, scale=1.0 / 16.0)
                                for hh in range(HD // 128):
                                    vsl = slice(h * HD + hh * 128,
                                                h * HD + (hh + 1) * 128)
                                    mm(atn_ps[hh][:], V[kc][:, vsl], PT[:],
                                       start=(kc == 0), stop=(kc == nkc - 1))
                                mm(rs_ps[:], onesq[:, :], PT[:],
                                   start=(kc == 0), stop=(kc == nkc - 1))
                            rcp = rcpp.tile([128, SC], fp32, tag="rcp")
                            # rowsums are in [1, ~2e7]: safe for the approx
                            nc.vector.reciprocal_approx_fast(rcp[:], rs_ps[:])
                            for hh in range(HD // 128):
                                nc.vector.tensor_tensor(
                                    ATN[hg * NHC + c0 + hh][:, qsl],
                                    atn_ps[hh][:], rcp[:],
                                    op=mybir.AluOpType.mult)

                    NAC = NHG * NHC      # 8 ATN chunks
                    if hg == 0:
                        with (
                            tc.tile_pool(name="ptsb", bufs=3) as ptp,
                            tc.tile_pool(name="rcp", bufs=2) as rcpp,
                            tc.tile_pool(name="scps", bufs=2, space="PSUM") as scps,
                            tc.tile_pool(name="atps", bufs=1, space="PSUM") as atps,
                            tc.tile_pool(name="rsps", bufs=1, space="PSUM") as rsps,
                        ):
                            for qm in range(NSC):
                                attn_qm((ptp, rcpp, scps, atps, rsps), 0, qm)
                    else:
                        # fused: attention(hg1, qm) -> out-proj rows 4qm..4qm+3
                        # -> ReduceScatter chunk qm (overlaps next qm compute)
                        with (
                            tc.tile_pool(name="ptsb", bufs=3) as ptp,
                            tc.tile_pool(name="rcp", bufs=2) as rcpp,
                            tc.tile_pool(name="wo", bufs=1) as wop,
                            tc.tile_pool(name="ysb", bufs=4) as ysbp,
                            tc.tile_pool(name="scps", bufs=2, space="PSUM") as scps,
                            tc.tile_pool(name="atps", bufs=1, space="PSUM") as atps,
                            tc.tile_pool(name="rsps", bufs=1, space="PSUM") as rsps,
                            tc.tile_pool(name="yps", bufs=3, space="PSUM") as ypsp,
                        ):
                            woc = [wop.tile([128, NAC, SC], mdt, tag=f"woc{oc}",
                                            name=f"woc{oc}") for oc in range(NOC)]
                            for oc in range(NOC):
                                nc.sync.dma_start(woc[oc][:], wo_e[oc])
                            dst = y_part if use_collective else y_e
                            for qm in range(NSC):
                                attn_qm((ptp, rcpp, scps, atps, rsps), 1, qm)
                                for sg in range(qm * 4, qm * 4 + 4):
                                    sgl = slice(sg * 128, (sg + 1) * 128)
                                    ysb = ysbp.tile([128, D], mdt, tag="ysb")
                                    for oc in range(NOC):
                                        yps = ypsp.tile([128, SC], fp32,
                                                        tag="yps")
                                        for c in range(NAC):
                                            mm(yps[:], ATN[c][:, sgl],
                                               woc[oc][:, c, :],
                                               start=(c == 0),
                                               stop=(c == NAC - 1))
                                        osl = slice(oc * SC, (oc + 1) * SC)
                                        if oc % 2 == 0:
                                            nc.vector.tensor_copy(
                                                ysb[:, osl], yps[:])
                                        else:
                                            nc.scalar.copy(ysb[:, osl], yps[:])
                                    nc.sync.dma_start(dst[sgl, :], ysb[:])
                                    if use_collective:
                                        for (r0, r1) in RS_CHUNKS:
                                            if r1 != (sg + 1) * 128:
                                                continue
                                            s0, s1 = r0 // GRP, r1 // GRP
                                            nc.gpsimd.collective_compute(
                                                "ReduceScatter",
                                                mybir.AluOpType.add,
                                                replica_groups=rgroups,
                                                ins=[y_part[r0:r1, :]],
                                                outs=[rs_out[s0:s1, :]],
                                            )
                                            nc.sync.dma_start(
                                                y_e[s0:s1, :],
                                                rs_out[s0:s1, :])

                    if debug_taps:
                        for c in range(NHC):
                            nc.sync.dma_start(dbg["dbg_qt"][hg, c], QT[c][:])
                            nc.sync.dma_start(dbg["dbg_kt"][hg, c], KT[c][:])
                        for ss in range(S // 128):
                            nc.sync.dma_start(
                                dbg["dbg_v"][hg, ss * 128:(ss + 1) * 128, :],
                                V[ss][:])

    nc.compile()
    return nc


# ---------------------------------------------------------------- host prep

def _sinusoidal_np(num_pos, dim):
    inv_freq = 1.0 / (10000.0 ** (np.arange(0, dim, 2, dtype=np.float32) / dim))
    t = np.arange(num_pos, dtype=np.float32)[:, None] * inv_freq[None, :]
    return np.cos(t).astype(np.float32), np.sin(t).astype(np.float32)  # [P, dim//2]


def _host_arrays(hs, Wq, Wk, Wv, Wo, position_ids, cfg=None, n_cores=N_CORES):
    """Build the shared + per-core input arrays (pre-tiled for DMA)."""
    import ml_dtypes
    bf16 = ml_dtypes.bfloat16

    hsT = hs.transpose(0, 2, 1).astype(bf16)                 # [B, D, S]
    # pre-tile hsT: [B][NG, 2, 128, GDC, S//2]
    hsT_t = np.ascontiguousarray(
        hsT.reshape(B, NG, GDC, 128, 2, S // 2).transpose(0, 1, 4, 3, 2, 5))

    cos_t, sin_t = _sinusoidal_np(MAX_POS, ROT)              # [P, ROT//2]
    pos = np.asarray(position_ids).astype(np.int64)          # [B, S]
    cosg = cos_t[pos]                                        # [B, S, 32]
    sing = sin_t[pos]
    cosb = np.repeat(cosg.transpose(0, 2, 1), 2, axis=1)     # [B, 64, S]
    sinb_r = np.repeat(sing.transpose(0, 2, 1), 2, axis=1)
    sgn = np.ones((ROT, 1), np.float32)
    sgn[0::2] = -1.0
    sinb = (sinb_r * sgn).astype(bf16)
    cosb = np.ascontiguousarray(cosb).astype(bf16)

    # transposed causal masks: mskT[kk, m, qq] = 0 if m*128+kk <= qq else NEG
    kk = np.arange(128)[:, None, None]
    m_ = np.arange(4)[None, :, None]
    qq = np.arange(SC)[None, None, :]
    mskT = np.where(m_ * 128 + kk <= qq, 0.0, NEG).astype(np.float32)
    mskT = np.ascontiguousarray(mskT)

    pswap = np.zeros((128, ROT), bf16)
    for f in range(ROT // 2):
        pswap[2 * f + 1, 2 * f] = 1.0
        pswap[2 * f, 2 * f + 1] = 1.0

    def tile_w(wT):       # [D, HDL] -> [NHG, NG, 128, GDC, HGW]
        return np.ascontiguousarray(
            wT.reshape(NG, GDC, 128, NHG, HGW).transpose(3, 0, 2, 1, 4))

    per_core = []
    for c in range(n_cores):
        beta, t = c // GRP, c % GRP
        csl = slice(t * HDL, (t + 1) * HDL)
        wqT = Wq[csl, :].T.astype(bf16)                      # [D, HDL]
        wkT = Wk[csl, :].T.astype(bf16)
        wvT = Wv[csl, :].T.astype(bf16)
        woT = Wo[:, csl].T.astype(bf16)                      # [HDL, D]
        wo_t = np.ascontiguousarray(
            woT.reshape(HDL // 128, 128, NOC, SC).transpose(2, 1, 0, 3))
        per_core.append(dict(
            hsT=hsT_t[beta],
            wq=tile_w(wqT), wk=tile_w(wkT), wv=tile_w(wvT), wo=wo_t,
            cosb=cosb[beta], sinb=sinb[beta],
            maskT=mskT, pswap=pswap,
        ))
    return per_core


RS_CHUNKS_HOST = [(0, 512), (512, 1024), (1024, 1536), (1536, 1792),
                  (1792, 1920), (1920, 2048)]


def assemble_output(outs, use_collective=True):
    """Reassemble per-core 'y' outputs into the full [B, S, D] fp32 result."""
    y = np.zeros((B, S, D), np.float32)
    for c, o in enumerate(outs):
        o = np.asarray(o, np.float32)        # [S//GRP, D] or [S, D]
        beta, t = c // GRP, c % GRP
        if use_collective:
            for (r0, r1) in RS_CHUNKS_HOST:
                L = (r1 - r0) // GRP
                y[beta, r0 + t * L:r0 + (t + 1) * L, :] = \
                    o[r0 // GRP:r0 // GRP + L]
        else:
            y[beta] += o
    return y


def _numpy_reference(hidden_states, Wq, Wk, Wv, Wo, layer_past_k, layer_past_v,
                     attention_mask, position_ids, new_key_loc, new_value_loc,
                     valid_key_indices, valid_value_indices, bucket_size):
    """Slow but general fallback (mirrors reference.py in numpy fp32)."""
    hs = np.asarray(hidden_states, np.float32)
    Bc, Sc, Dc = hs.shape
    q = (hs @ np.asarray(Wq).T).reshape(Bc, Sc, NH, HD)
    k = (hs @ np.asarray(Wk).T).reshape(Bc, Sc, NH, HD)
    v = (hs @ np.asarray(Wv).T).reshape(Bc, Sc, NH, HD)

    cos_t, sin_t = _sinusoidal_np(MAX_POS, ROT)
    pos = np.asarray(position_ids).astype(np.int64)
    c_ = cos_t[pos][:, :, None, :]      # [B,S,1,32]
    s_ = sin_t[pos][:, :, None, :]

    def rot(x):
        xr = x[..., :ROT].reshape(Bc, Sc, NH, ROT // 2, 2)
        x0, x1 = xr[..., 0], xr[..., 1]
        o0 = c_ * x0 - s_ * x1
        o1 = s_ * x0 + c_ * x1
        out = np.stack([o0, o1], axis=-1).reshape(Bc, Sc, NH, ROT)
        return np.concatenate([out, x[..., ROT:]], axis=-1)

    q, k = rot(q), rot(k)
    nk = np.asarray(layer_past_k, np.float32).copy()
    nv = np.asarray(layer_past_v, np.float32).copy()
    nk[np.asarray(new_key_loc)] = k.reshape(Bc * Sc, 1, NH, HD)
    nv[np.asarray(new_value_loc)] = v.reshape(Bc * Sc, 1, NH, HD)
    kg = nk[np.asarray(valid_key_indices)].reshape(
        Bc, bucket_size, NH, HD).transpose(0, 2, 1, 3)
    vg = nv[np.asarray(valid_value_indices)].reshape(
        Bc, bucket_size, NH, HD).transpose(0, 2, 1, 3)
    qh = q.transpose(0, 2, 1, 3)
    scores = np.einsum("bhqd,bhkd->bhqk", qh, kg)
    causal = np.tril(np.ones((MAX_POS, MAX_POS), bool))[
        bucket_size - Sc:bucket_size, :bucket_size]
    scores = np.where(causal, scores, np.float32(np.finfo(np.float32).min))
    scores = scores / np.float32(np.sqrt(HD)) + np.asarray(attention_mask, np.float32)
    scores = scores - scores.max(-1, keepdims=True)
    p = np.exp(scores)
    p = p / p.sum(-1, keepdims=True)
    attn = np.einsum("bhqk,bhkd->bhqd", p, vg)
    attn = attn.transpose(0, 2, 1, 3).reshape(Bc, Sc, Dc)
    return (attn @ np.asarray(Wo).T).astype(np.float32)


def _fast_path_ok(layer_past_k, layer_past_v, attention_mask, new_key_loc,
                  new_value_loc, valid_key_indices, valid_value_indices,
                  bucket_size, hs_shape):
    Bc, Sc, Dc = hs_shape
    if (Bc, Sc, Dc) != (B, S, D) or int(bucket_size) != S:
        return False
    ar = np.arange(Bc * Sc)
    for idx in (new_key_loc, new_value_loc, valid_key_indices, valid_value_indices):
        a = np.asarray(idx)
        if a.shape != (Bc * Sc,) or not np.array_equal(a, ar):
            return False
    if np.any(np.asarray(attention_mask) != 0):
        return False
    return True


_NC_CACHE = {}


def _get_nc(use_collective=True):
    key = ("v2", use_collective)
    if key not in _NC_CACHE:
        _NC_CACHE[key] = build_nc(use_collective=use_collective,
                                  n_cores=N_CORES)
    return _NC_CACHE[key]


def kernel(**inputs):
    hs = np.asarray(inputs["hidden_states"], np.float32)
    fast = _fast_path_ok(
        inputs["layer_past_k"], inputs["layer_past_v"], inputs["attention_mask"],
        inputs["new_key_loc"], inputs["new_value_loc"],
        inputs["valid_key_indices"], inputs["valid_value_indices"],
        inputs["bucket_size"], hs.shape)
    if not fast:
        return _numpy_reference(**inputs)

    from concourse.bass_utils import run_bass_kernel_spmd

    use_collective = os.environ.get("KERNEL_NO_COLLECTIVE", "") != "1"
    nc = _get_nc(use_collective)
    in_maps = _host_arrays(
        hs, np.asarray(inputs["Wq"], np.float32),
        np.asarray(inputs["Wk"], np.float32),
        np.asarray(inputs["Wv"], np.float32),
        np.asarray(inputs["Wo"], np.float32),
        inputs["position_ids"])
    res = run_bass_kernel_spmd(nc, in_maps, list(range(N_CORES)))
    outs = [res.results[c]["y"] for c in range(N_CORES)]
    return assemble_output(outs, use_collective)
